# revision 44
# baseline (speedup 1.0000x reference)
"""Trainium2 Bass kernel for nn_FAttention1d (attention with softmax over the
QUERY axis).

Reference computation (B=2, H=16, S=2048, D=64, fp32):
    att[b,h,q,k] = sum_d qry[b,h,q,d] * key[b,h,k,d]
    att += reg * I_S                      (diagonal in (q,k))
    att = softmax(att, axis=q)            (normalize over the QUERY axis)
    out[b,h,q,v] = sum_k att[b,h,q,k] * val[b,h,k,v]

Sharding: the 32 (b,h) pairs are split 4-per-core across 8 NeuronCores; the
whole S=2048 attention chain is local to a core.

Device-side layout: compute S^T = K @ Q^T with k on the partition axis, so the
q-axis softmax is a free-axis reduction (fused into the exp pass via the ACT
accumulator), and exp(S^T) tiles feed the A^T V matmul directly as the moving
operand:
    out^T[v,q] = sum_k (val[k,v] / r[k])_stationary @ exp(S^T)[k,q]
with r[k] = sum_q exp(S^T[k,q]) folded into the val rows.

Engine balance (per core ~16.8M exp elements move PSUM->SBUF):
  - ACT: all exp ops. A minority of (head, k-tile) score tiles ("direct") are
    exp'd straight from PSUM in two [128,1024] ops; the rest are staged to
    SBUF by DVE and exp'd in one wide [128,2048] op (cheaper per column for
    ACT, costs DVE a copy). The direct count balances ACT vs DVE.
  - DVE: staging copies, PSUM diag adds for direct tiles, reciprocal, out^T
    PSUM->SBUF copies.
  - Pool: diag adds on staged SBUF tiles, r half merges, val/r scaling.
  - PE: QK^T and AV matmuls, AV drained between QK chunks so PE follows the
    ACT/DVE pacemakers without bursts.
PSUM: out^T accumulator [128,2048] (4 banks) + 2x [128,1024] score tiles.
"""

import numpy as np
from collections import deque
from contextlib import ExitStack

import concourse.bass as bass
import concourse.mybir as mybir
import concourse.tile as tile
from concourse import bacc
from concourse.bass_utils import run_bass_kernel_spmd

B, H, S, D = 2, 16, 2048, 64
N_CORES = 8
BH = B * H                     # 32
BH_PER_CORE = BH // N_CORES    # 4
NT = S // 128                  # 16 k-tiles of 128
F32 = mybir.dt.float32
F16 = mybir.dt.float16
BF16 = mybir.dt.bfloat16

# Some (pair, k-tile, head) score tiles are exp'd directly from PSUM (2x
# [128,1024] ACT ops); the rest are DVE-staged to SBUF and exp'd in one
# [128,2048] ACT op (cheaper per column for ACT, costs DVE the copies).
# DIRECT_UNITS[p] holds (n, s) units exp'd direct for pair p. The mix
# balances ACT (wide exps + direct exps) against DVE (staging copies);
# direct-heavy at kernel start (ACT has no backlog), staged-heavy at pair
# tails so deferred wides keep ACT fed across the pair boundary.
DIRECT_NS = set()
# ~50 staged / 14 direct units: on HW each exp op costs an extra ~284ns
# ACTIVATION_READ_ACCUMULATOR, so wide staged exps (1 accum read / 2048
# cols) are cheaper for ACT than direct exps (1 per 1024); DVE absorbs the
# staging copies with slack to spare once PE stops pacing.
_D6 = {(n, 0) for n in [2, 5, 8, 11, 13, 15]}
DIRECT_UNITS = {0: set(_D6), 1: _D6 | {(14, 0), (14, 1), (15, 1)}}


def _is_direct(p, n, s):
    if DIRECT_UNITS:
        return (n, s) in DIRECT_UNITS.get(p, set())
    return s == 0 and n in DIRECT_NS


# "X" score tiles stage only the h0 half (one DVE copy); the h1 half is
# exp'd straight from PSUM. Both exps are [128,1024] ops, short enough to
# meet the PSUM ping-pong reuse window even behind one queued wide exp.
# Trades ~1.2us DVE for ~0.4us ACT per tile to balance the two pacemakers.
X_NS = set()


def _is_x(p, n, s):
    return n in X_NS


def _build_kernel(nc, tc, ctx, qt, kt, vs, rg, rgb, ot):
    # bufs=2: with a single buffer, the next iteration's rg/rgb DMAs wait on
    # THIS iteration's last diag reads, head-of-line-blocking the sync DMA
    # queue and stalling the q2/k2 prefetch behind them (~10us ramp/iter)
    const_pool = ctx.enter_context(tc.tile_pool(name="const", bufs=2))
    q_pool = ctx.enter_context(tc.tile_pool(name="q", bufs=2))
    k_pool = ctx.enter_context(tc.tile_pool(name="k", bufs=2))
    v_pool = ctx.enter_context(tc.tile_pool(name="v", bufs=2))
    e_pool = ctx.enter_context(tc.tile_pool(name="e", bufs=8))
    stg_pool = ctx.enter_context(tc.tile_pool(name="stg", bufs=4))
    r_pool = ctx.enter_context(tc.tile_pool(name="r", bufs=2))
    vsc_pool = ctx.enter_context(tc.tile_pool(name="vsc", bufs=6))
    osb_pool = ctx.enter_context(tc.tile_pool(name="osb", bufs=2))
    st_pool = ctx.enter_context(tc.tile_pool(name="st", bufs=2, space="PSUM"))
    o_pool = ctx.enter_context(tc.tile_pool(name="o", bufs=1, space="PSUM"))

    rg_eye = const_pool.tile([128, 128], F32)
    nc.sync.dma_start(rg_eye[:], rg[:])
    # [I, reg*I] as bf16 for the PE diag-accumulate matmul (I.T @ reg*I adds
    # reg to the score diagonal in PSUM, costing PE ~60ns instead of a DVE op)
    eyb = const_pool.tile([128, 256], BF16)
    nc.sync.dma_start(eyb[:], rgb[:])
    # warm the ACT Exp table during the input DMA so the first real exp
    # doesn't pay the 1.3us table load
    warm = const_pool.tile([128, 1], F32)
    nc.scalar.activation(warm[:], rg_eye[:, 0:1],
                         mybir.ActivationFunctionType.Exp)
    # (PE warmup burst removed: faulted on HW)

    AB = (0, 1)
    # Prefetch ALL pairs' inputs up front (pools are double-buffered, both
    # pairs fit). If pair p+1's input DMAs were issued inside its pair body,
    # they would queue BEHIND pair p's output DMAs on the shared gpsimd
    # HWDGE queue and stall the next pair's QK chain ~6us at the boundary.
    # ramp-critical inputs (k2, q2) ride the sync HWDGE queue, which carries
    # no output DMAs — so the NEXT iteration's prefetch is never stuck
    # behind this iteration's outs. v (needed only from n~4) + all outs ride
    # the gpsimd SWDGE queue.
    q2s, k2s, vsbs = [], [], []
    for p in range(BH_PER_CORE // 2):
        q2 = q_pool.tile([128, S], F16, tag="q2", name="q2")
        k2 = k_pool.tile([128, S], F16, tag="k2", name="k2")
        nc.sync.dma_start(k2[:, 0:128], kt[p][:, 0:128])
        nc.sync.dma_start(q2[:, 0:512], qt[p][:, 0:512])
        nc.sync.dma_start(q2[:, 512:1024], qt[p][:, 512:1024])
        nc.sync.dma_start(k2[:, 128:], kt[p][:, 128:])
        nc.sync.dma_start(q2[:, 1024:], qt[p][:, 1024:])
        v_sb = [None, None]
        for s in AB:
            v_sb[s] = v_pool.tile([128, NT * 64], F32, tag=f"v{s}", name=f"v_sb{s}")
            nc.gpsimd.dma_start(v_sb[s][:], vs[2 * p + s])
        q2s.append(q2)
        k2s.append(k2)
        vsbs.append(v_sb)

    for p in range(BH_PER_CORE // 2):
        bh = (2 * p, 2 * p + 1)
        q2, k2, v_sb = q2s[p], k2s[p], vsbs[p]

        # out^T for the pair: partitions 0-63 = bh A, 64-127 = bh B
        o_ps = o_pool.tile([128, S], F32)
        r_all = [r_pool.tile([128, 2, NT], F32, tag=f"rall{s}", name=f"r_all{s}") for s in AB]
        r_sum = [r_pool.tile([128, NT], F32, tag=f"rsum{s}", name=f"r_sum{s}") for s in AB]
        r_inv = [r_pool.tile([128, NT], F32, tag=f"rinv{s}", name=f"r_inv{s}") for s in AB]
        e_tiles = [[None] * NT, [None] * NT]
        vsc_tiles = [[None] * NT, [None] * NT]
        pending = deque()

        def queue_av_tiles(ms):
            # enqueue col-packed AV matmuls for k-tiles ms; drained a few at a
            # time between QK chunks so PE tracks the ACT/DVE pacemakers
            for m in ms:
                for ch in range(4):
                    pending.append((m, ch))

        out_sb = osb_pool.tile([128, S], F32)

        evac_q = deque()

        def emit_evac():
            while evac_q:
                ch = evac_q.popleft()
                # evac on ACT: DVE carries the staging copies + diag adds
                # (heavier engine); ScalarE reads PSUM slightly faster
                nc.scalar.copy(out_sb[:, ch], o_ps[:, ch])
                # both output DMAs on the gpsimd queue — keeping the sync
                # queue free for the next iteration's k2/q2 prefetch
                nc.gpsimd.dma_start(ot[bh[0]][:, ch],
                                    out_sb[0:64, ch])
                nc.gpsimd.dma_start(ot[bh[1]][:, ch],
                                    out_sb[64:128, ch])

        def drain_pending(k=2):
            for _ in range(k):
                if not pending:
                    emit_evac()
                    return
                # don't pop an AV chunk whose vsc isn't emitted yet
                if any(vsc_tiles[s][pending[0][0]] is None for s in AB):
                    return
                m, h = pending.popleft()
                ch = slice(h * 512, (h + 1) * 512)
                for s in AB:
                    # bh A -> out partitions 0-63, bh B -> 64-127
                    nc.tensor.matmul(
                        o_ps[64 * s:64 * s + 64, ch],
                        lhsT=vsc_tiles[s][m][:],
                        rhs=e_tiles[s][m][:, ch],
                        start=(m == 0),
                        stop=(m == NT - 1),
                        skip_group_check=True,
                    )
                if m == NT - 1:
                    # last accumulation for this q-chunk: queue the
                    # evacuation copy + DMA, emitted AFTER the next drain
                    # batch's matmuls so the tail doesn't alternate
                    # PE-MM / DVE-copy serially.
                    evac_q.append(ch)
            emit_evac()

        def emit_vsc(ms):
            # r_inv on DVE (cheap [128,few] op); vsc[m] = val * r_inv[m] on
            # ACT as a Copy with a per-partition scale (same act table as
            # Exp, no table switch). Pool/gpsimd compute ops are avoided
            # entirely: their real Q7 launch overhead is micro-seconds, not
            # the ~100ns the cost model charges.
            if not ms:
                return
            for s in AB:
                nc.vector.reciprocal_approx_fast(
                    r_inv[s][:, ms[0]:ms[-1] + 1],
                    r_sum[s][:, ms[0]:ms[-1] + 1])
            for m in ms:
                for s in AB:
                    vsc = vsc_pool.tile([128, 64], BF16, tag=f"vsc{s}",
                                        name=f"vsc{s}")
                    vsc_tiles[s][m] = vsc
                    nc.vector.tensor_scalar_mul(
                        vsc[:], v_sb[s][:, m * 64:(m + 1) * 64],
                        r_inv[s][:, m:m + 1],
                    )

        # vsc for tile m is emitted inside tile VSC_AT[m] (after that tile's
        # diag units, so Pool never gates the diag->exp chain); its AV
        # matmuls are queued one tile later so PE never waits on fresh vsc.
        # m=14 can be vsc'd/drained during n=15 only when n=14's exps happen
        # in-loop (fully direct); a staged n=14 wide is flushed after n=15
        # and would stall PE's AV(14) matmuls mid-loop.
        tail14 = all(_is_direct(p, 14, s) for s in AB)
        VSC_AT = {4: [0, 1, 2, 3], 8: [4, 5, 6, 7], 12: [8, 9, 10, 11],
                  14: [12], 15: [13, 14] if tail14 else [13]}
        QUEUE_AT = {5: [0, 1, 2, 3], 9: [4, 5, 6, 7], 13: [8, 9, 10, 11],
                    14: [12], 15: [13, 14] if tail14 else [13]}
        TAIL_MS = [15] if tail14 else [14, 15]

        # Wide (staged) exps are deferred by one full n-iteration: ACT's
        # queue is strict FIFO, so a wide exp whose stage input isn't copied
        # yet blocks later PSUM-freeing direct exps and stalls PE+DVE. With
        # a one-iteration delay the staging copies are certainly done and
        # ACT never idles mid-queue.
        deferred_wides = []
        deferred_prev = []

        def flush_prev_wides():
            nonlocal deferred_prev
            for fn in deferred_prev:
                fn()
            deferred_prev = []

        def flush_wides(all_pending=False):
            nonlocal deferred_prev
            flush_prev_wides()
            deferred_prev = deferred_wides[:]
            deferred_wides.clear()
            if all_pending:
                flush_prev_wides()

        def emit_r_merge_and_vsc(ms):
            # issue-order correctness: the reciprocal below READS r_sum[m];
            # a deferred wide exp for m issued later would leave the read
            # uninitialized (Tile deps follow issue order). Flush the
            # previous iteration's wides first — they cover every staged
            # m <= n-1 consumed here.
            if ms:
                flush_prev_wides()
            for m in ms:
                for s in AB:
                    if _is_direct(p, m, s) or _is_x(p, m, s):
                        nc.vector.tensor_add(
                            r_sum[s][:, m:m + 1], r_all[s][:, 0, m:m + 1],
                            r_all[s][:, 1, m:m + 1]
                        )
            emit_vsc(ms)

        for n in range(NT):
            hd = n // 8               # q-half containing this tile's diagonal
            cd = (n % 8) * 128        # diag column offset within that half
            queue_av_tiles(QUEUE_AT.get(n, []))
            for s in AB:
                e_tiles[s][n] = e_pool.tile([128, S], BF16, tag=f"e{s}",
                                            name=f"e{s}_{n}")
            stage = [None if _is_direct(p, n, s) else
                     stg_pool.tile(
                         [128, 1024 if _is_x(p, n, s) else S], F32,
                         tag=f"stg{s}" + ("x" if _is_x(p, n, s) else ""),
                         name=f"stage{s}")
                     for s in AB]
            for h in range(2):
                # QK matmuls interleaved across s (row groups 0-63 / 64-127):
                # consecutive same-weight MMs serialize (the repeated
                # LDWEIGHTS must wait for the in-flight MM on the same cells
                # to drain), but alternating row groups lets each LDWEIGHTS
                # pull ahead during the other head's MM — near 2x PE QK rate.
                sts = {}
                for s in AB:
                    sts[s] = st_pool.tile([128, 1024], F32, tag="st",
                                          name=f"st{s}")
                for j in range(2):
                    for s in AB:
                        q0 = h * 1024 + j * 512
                        nc.tensor.matmul(
                            sts[s][:, j * 512:(j + 1) * 512],
                            lhsT=k2[64 * s:64 * s + 64, n * 128:(n + 1) * 128],
                            rhs=q2[64 * s:64 * s + 64, q0:q0 + 512],
                            start=True,
                            stop=True,
                        )
                for s in AB:
                    direct = _is_direct(p, n, s)
                    st = sts[s]
                    if direct:
                        if h == hd:
                            # diag add in PSUM on PE (same trick as staged)
                            nc.tensor.matmul(
                                st[:, cd:cd + 128],
                                lhsT=eyb[:, 0:128],
                                rhs=eyb[:, 128:256],
                                start=False,
                                stop=True,
                                skip_group_check=True,
                            )
                        nc.scalar.activation(
                            e_tiles[s][n][:, h * 1024:(h + 1) * 1024],
                            st[:],
                            mybir.ActivationFunctionType.Exp,
                            accum_out=r_all[s][:, h:h + 1, n:n + 1],
                        )
                    elif _is_x(p, n, s):
                        if h == 0:
                            nc.vector.tensor_copy(
                                stage[s][:, 0:1024], st[:])
                            if hd == 0:
                                nc.vector.tensor_add(
                                    stage[s][:, cd:cd + 128],
                                    stage[s][:, cd:cd + 128],
                                    rg_eye[:])
                        else:
                            if hd == 1:
                                nc.tensor.matmul(
                                    st[:, cd:cd + 128],
                                    lhsT=eyb[:, 0:128],
                                    rhs=eyb[:, 128:256],
                                    start=False,
                                    stop=True,
                                    skip_group_check=True,
                                )
                            # PSUM-freeing exp first; the SBUF-side exp is
                            # deferred with the wides
                            nc.scalar.activation(
                                e_tiles[s][n][:, 1024:2048],
                                st[:],
                                mybir.ActivationFunctionType.Exp,
                                accum_out=r_all[s][:, 1:2, n:n + 1],
                            )

                            def _x_exp(s=s, n=n, stg=stage[s]):
                                nc.scalar.activation(
                                    e_tiles[s][n][:, 0:1024],
                                    stg[:, 0:1024],
                                    mybir.ActivationFunctionType.Exp,
                                    accum_out=r_all[s][:, 0:1, n:n + 1],
                                )
                            deferred_wides.append(_x_exp)
                    else:
                        # staged: diag added on the SBUF stage by DVE after
                        # the copy — keeps PE's fp16 QK weight stream free of
                        # eyb (bf16) LDWEIGHTS churn
                        nc.vector.tensor_copy(
                            stage[s][:, h * 1024:(h + 1) * 1024], st[:])
                        if h == hd:
                            gc = hd * 1024 + cd
                            nc.vector.tensor_add(
                                stage[s][:, gc:gc + 128],
                                stage[s][:, gc:gc + 128],
                                rg_eye[:])
                        if h == 1:
                            def _wide_exp(s=s, n=n, stg=stage[s]):
                                nc.scalar.activation(
                                    e_tiles[s][n][:],
                                    stg[:],
                                    mybir.ActivationFunctionType.Exp,
                                    accum_out=r_sum[s][:, n:n + 1],
                                )
                            deferred_wides.append(_wide_exp)
                    if s == 1 and h == hd:
                        emit_r_merge_and_vsc(VSC_AT.get(n, []))
                    # drain AV in moderate batches (twice per n): big
                    # once-per-n bursts starve the QK/consumer pipeline
                    # (+44us on HW); 1-2 MM dribbles keep HAM cold
                    if s == 1:
                        drain_pending(2 if n < 6 else (4 if n < 15 else 6))
            flush_wides(all_pending=(n == NT - 1))
        emit_r_merge_and_vsc(TAIL_MS)
        queue_av_tiles(TAIL_MS)
        while pending:
            drain_pending(4)


_NC_CACHE = {}


def build_nc(repeats=1):
    key = repeats
    if key in _NC_CACHE:
        return _NC_CACHE[key]
    nc = bacc.Bacc("TRN2", target_bir_lowering=False, debug=False)
    qt = nc.dram_tensor("qt", [BH_PER_CORE // 2, 2 * D, S], F16, kind="ExternalInput").ap()
    kt = nc.dram_tensor("kt", [BH_PER_CORE // 2, 2 * D, S], F16, kind="ExternalInput").ap()
    vs = nc.dram_tensor("vs", [BH_PER_CORE, 128, NT * 64], F32, kind="ExternalInput").ap()
    rg = nc.dram_tensor("rg", [128, 128], F32, kind="ExternalInput").ap()
    rgb = nc.dram_tensor("rgb", [128, 256], BF16, kind="ExternalInput").ap()
    ot = nc.dram_tensor("ot", [BH_PER_CORE, D, S], F32, kind="ExternalOutput").ap()
    with tile.TileContext(nc) as tc, ExitStack() as ctx:
        if repeats == 1:
            _build_kernel(nc, tc, ctx, qt, kt, vs, rg, rgb, ot)
        else:
            # benchmarking mode: repeat the whole kernel body in an on-device
            # loop so per-iteration time can be extracted from wall clock.
            # A pre-loop exp forces the ACT Exp table load OUTSIDE the loop
            # (otherwise walrus re-inserts the ~2.7us PSEUDO_LOAD per
            # iteration).
            pre_pool = ctx.enter_context(tc.tile_pool(name="pre", bufs=1))
            pre = pre_pool.tile([128, 2], F32)
            nc.sync.dma_start(pre[:, 0:1], rg[:, 0:1])
            nc.scalar.activation(pre[:, 1:2], pre[:, 0:1],
                                 mybir.ActivationFunctionType.Exp)
            with tc.For_i(0, repeats, 1,
                          hint_engines=(mybir.EngineType.PE,
                                        mybir.EngineType.Activation,
                                        mybir.EngineType.DVE)):
                _build_kernel(nc, tc, ctx, qt, kt, vs, rg, rgb, ot)
    nc.compile()
    _NC_CACHE[key] = nc
    return nc


def _prep_inputs(qry, key, val, reg):
    """Host-side shard + layout prep. Returns per-core input maps."""
    q = np.ascontiguousarray(np.asarray(qry, dtype=np.float32)).reshape(BH, S, D)
    k = np.ascontiguousarray(np.asarray(key, dtype=np.float32)).reshape(BH, S, D)
    v = np.ascontiguousarray(np.asarray(val, dtype=np.float32)).reshape(BH, S, D)
    rg = (np.eye(128, dtype=np.float32) * np.float32(np.asarray(reg)))
    import ml_dtypes
    rgb = np.concatenate([np.eye(128, dtype=np.float32),
                          np.eye(128, dtype=np.float32)
                          * np.float32(np.asarray(reg))],
                         axis=1).astype(ml_dtypes.bfloat16)

    in_maps = []
    for c in range(N_CORES):
        sl = slice(c * BH_PER_CORE, (c + 1) * BH_PER_CORE)
        qt = np.ascontiguousarray(
            q[sl].transpose(0, 2, 1).reshape(BH_PER_CORE // 2, 2 * D, S)
        ).astype(np.float16)                                          # [2, 128, S]
        kt = np.ascontiguousarray(
            k[sl].transpose(0, 2, 1).reshape(BH_PER_CORE // 2, 2 * D, S)
        ).astype(np.float16)                                          # [2, 128, S]
        vv = v[sl].reshape(BH_PER_CORE, NT, 128, D)
        vs = np.ascontiguousarray(vv.transpose(0, 2, 1, 3)).reshape(
            BH_PER_CORE, 128, NT * D)                                 # [4, 128, 1024]
        in_maps.append({"qt": qt, "kt": kt, "vs": vs, "rg": rg, "rgb": rgb})
    return in_maps


def kernel(qry, key, val, reg):
    nc = build_nc()
    in_maps = _prep_inputs(qry, key, val, reg)
    res = run_bass_kernel_spmd(nc, in_maps, list(range(N_CORES)))
    out = np.empty((BH, S, D), dtype=np.float32)
    for c in range(N_CORES):
        ot = res.results[c]["ot"]                                    # [4, 64, S]
        for i in range(BH_PER_CORE):
            out[c * BH_PER_CORE + i] = ot[i].T
    return out.reshape(B, H, S, D)



# revision 49
# speedup vs baseline: 1.0203x; 1.0203x over previous
"""Trainium2 Bass kernel for nn_FAttention1d (attention with softmax over the
QUERY axis).

Reference computation (B=2, H=16, S=2048, D=64, fp32):
    att[b,h,q,k] = sum_d qry[b,h,q,d] * key[b,h,k,d]
    att += reg * I_S                      (diagonal in (q,k))
    att = softmax(att, axis=q)            (normalize over the QUERY axis)
    out[b,h,q,v] = sum_k att[b,h,q,k] * val[b,h,k,v]

Sharding: the 32 (b,h) pairs are split 4-per-core across 8 NeuronCores; the
whole S=2048 attention chain is local to a core.

Device-side layout: compute S^T = K @ Q^T with k on the partition axis, so the
q-axis softmax is a free-axis reduction (fused into the exp pass via the ACT
accumulator), and exp(S^T) tiles feed the A^T V matmul directly as the moving
operand:
    out^T[v,q] = sum_k (val[k,v] / r[k])_stationary @ exp(S^T)[k,q]
with r[k] = sum_q exp(S^T[k,q]) folded into the val rows.

Engine balance (per core ~16.8M exp elements move PSUM->SBUF; HW-profiled
at ~147us busy on each of PE / ACT / DVE):
  - ACT: all exp ops (each also pays a ~284ns ACTIVATION_READ_ACCUMULATOR
    for its r accum). A minority of (head, k-tile) score units ("direct",
    DIRECT_UNITS) are exp'd straight from PSUM in two [128,1024] ops; the
    rest are staged to SBUF by DVE and exp'd in one wide [128,2048] op
    (cheaper per column AND one accum read per 2048 cols). Wide exps are
    issue-deferred one k-tile so ACT's strict FIFO never blocks a
    PSUM-freeing direct exp behind a wide whose staging isn't done. Also
    the out^T PSUM->SBUF evacuation copies.
  - DVE: staging copies (1x mode, PSUM fp32 src), diag adds on the SBUF
    stage, r half merges, reciprocal, val/r scaling.
  - PE: QK^T and AV matmuls. QK j-chunks are interleaved across the two
    heads (row groups 0-63/64-127) so every LDWEIGHTS targets the row
    group NOT in flight and pulls ahead — consecutive same-cell reloads
    would serialize each MM at its full isolated latency. Direct-unit diag
    adds ride PE as an I.T @ reg*I accumulate; staged diags go to DVE to
    keep the fp16 QK weight stream clean. AV is drained in moderate
    batches (2x per k-tile) — bigger bursts starve the consumer pipeline.
PSUM: out^T accumulator [128,2048] (4 banks) + 2x [128,1024] score tiles.
Issue-order rule: Tile deps follow issue order — any deferred producer
(wide exp) MUST be flushed before a consumer of its output (reciprocal of
r_sum) is issued, or the consumer reads garbage nondeterministically.
"""

import numpy as np
from collections import deque
from contextlib import ExitStack

import concourse.bass as bass
import concourse.mybir as mybir
import concourse.tile as tile
from concourse import bacc
from concourse.bass_utils import run_bass_kernel_spmd

B, H, S, D = 2, 16, 2048, 64
N_CORES = 8
BH = B * H                     # 32
BH_PER_CORE = BH // N_CORES    # 4
NT = S // 128                  # 16 k-tiles of 128
F32 = mybir.dt.float32
F16 = mybir.dt.float16
BF16 = mybir.dt.bfloat16

# Some (pair, k-tile, head) score tiles are exp'd directly from PSUM (2x
# [128,1024] ACT ops); the rest are DVE-staged to SBUF and exp'd in one
# [128,2048] ACT op (cheaper per column for ACT, costs DVE the copies).
# DIRECT_UNITS[p] holds (n, s) units exp'd direct for pair p. The mix
# balances ACT (wide exps + direct exps) against DVE (staging copies);
# direct-heavy at kernel start (ACT has no backlog), staged-heavy at pair
# tails so deferred wides keep ACT fed across the pair boundary.
DIRECT_NS = set()
# ~50 staged / 14 direct units: on HW each exp op costs an extra ~284ns
# ACTIVATION_READ_ACCUMULATOR, so wide staged exps (1 accum read / 2048
# cols) are cheaper for ACT than direct exps (1 per 1024); DVE absorbs the
# staging copies with slack to spare once PE stops pacing.
_D6 = {(n, 0) for n in [2, 5, 8, 11, 13, 15]}
DIRECT_UNITS = {0: set(_D6), 1: _D6 | {(14, 0), (14, 1), (15, 1)}}


def _is_direct(p, n, s):
    if DIRECT_UNITS:
        return (n, s) in DIRECT_UNITS.get(p, set())
    return s == 0 and n in DIRECT_NS


# "X" score tiles stage only the h0 half (one DVE copy); the h1 half is
# exp'd straight from PSUM. Both exps are [128,1024] ops, short enough to
# meet the PSUM ping-pong reuse window even behind one queued wide exp.
# Trades ~1.2us DVE for ~0.4us ACT per tile to balance the two pacemakers.
X_NS = set()


def _is_x(p, n, s):
    return n in X_NS


def _build_kernel(nc, tc, ctx, qt, kt, vs, rg, rgb, ot):
    # bufs=2: with a single buffer, the next iteration's rg/rgb DMAs wait on
    # THIS iteration's last diag reads, head-of-line-blocking the sync DMA
    # queue and stalling the q2 prefetch behind them
    const_pool = ctx.enter_context(tc.tile_pool(name="const", bufs=2))
    q_pool = ctx.enter_context(tc.tile_pool(name="q", bufs=2))
    k_pool = ctx.enter_context(tc.tile_pool(name="k", bufs=2))
    v_pool = ctx.enter_context(tc.tile_pool(name="v", bufs=2))
    e_pool = ctx.enter_context(tc.tile_pool(name="e", bufs=8))
    stg_pool = ctx.enter_context(tc.tile_pool(name="stg", bufs=4))
    r_pool = ctx.enter_context(tc.tile_pool(name="r", bufs=2))
    vsc_pool = ctx.enter_context(tc.tile_pool(name="vsc", bufs=6))
    osb_pool = ctx.enter_context(tc.tile_pool(name="osb", bufs=2))
    st_pool = ctx.enter_context(tc.tile_pool(name="st", bufs=2, space="PSUM"))
    o_pool = ctx.enter_context(tc.tile_pool(name="o", bufs=1, space="PSUM"))

    rg_eye = const_pool.tile([128, 128], F32)
    nc.sync.dma_start(rg_eye[:], rg[:])
    # [I, reg*I] as bf16 for the PE diag-accumulate matmul (I.T @ reg*I adds
    # reg to the score diagonal in PSUM, costing PE ~60ns instead of a DVE op)
    eyb = const_pool.tile([128, 256], BF16)
    nc.sync.dma_start(eyb[:], rgb[:])
    # warm the ACT Exp table during the input DMA so the first real exp
    # doesn't pay the 1.3us table load
    warm = const_pool.tile([128, 1], F32)
    nc.scalar.activation(warm[:], rg_eye[:, 0:1],
                         mybir.ActivationFunctionType.Exp)
    # (PE warmup burst removed: faulted on HW)

    AB = (0, 1)
    # Prefetch ALL pairs' inputs up front (pools are double-buffered, both
    # pairs fit). If pair p+1's input DMAs were issued inside its pair body,
    # they would queue BEHIND pair p's output DMAs on the shared gpsimd
    # HWDGE queue and stall the next pair's QK chain ~6us at the boundary.
    # ramp-critical inputs (k2, q2) ride the sync HWDGE queue, which carries
    # no output DMAs — so the NEXT iteration's prefetch is never stuck
    # behind this iteration's outs. v (needed only from n~4) + all outs ride
    # the gpsimd SWDGE queue.
    q2s, k2s, vsbs = [], [], []
    for p in range(BH_PER_CORE // 2):
        q2 = q_pool.tile([128, S], F16, tag="q2", name="q2")
        k2 = k_pool.tile([128, S], F16, tag="k2", name="k2")
        nc.gpsimd.dma_start(k2[:, 0:128], kt[p][:, 0:128])
        nc.sync.dma_start(q2[:, 0:512], qt[p][:, 0:512])
        nc.sync.dma_start(q2[:, 512:1024], qt[p][:, 512:1024])
        nc.gpsimd.dma_start(k2[:, 128:], kt[p][:, 128:])
        nc.sync.dma_start(q2[:, 1024:], qt[p][:, 1024:])
        v_sb = [None, None]
        for s in AB:
            v_sb[s] = v_pool.tile([128, NT * 64], F32, tag=f"v{s}", name=f"v_sb{s}")
            nc.gpsimd.dma_start(v_sb[s][:], vs[2 * p + s])
        q2s.append(q2)
        k2s.append(k2)
        vsbs.append(v_sb)

    for p in range(BH_PER_CORE // 2):
        bh = (2 * p, 2 * p + 1)
        q2, k2, v_sb = q2s[p], k2s[p], vsbs[p]

        # out^T for the pair: partitions 0-63 = bh A, 64-127 = bh B
        o_ps = o_pool.tile([128, S], F32)
        r_all = [r_pool.tile([128, 2, NT], F32, tag=f"rall{s}", name=f"r_all{s}") for s in AB]
        r_sum = [r_pool.tile([128, NT], F32, tag=f"rsum{s}", name=f"r_sum{s}") for s in AB]
        r_inv = [r_pool.tile([128, NT], F32, tag=f"rinv{s}", name=f"r_inv{s}") for s in AB]
        e_tiles = [[None] * NT, [None] * NT]
        vsc_tiles = [[None] * NT, [None] * NT]
        pending = deque()

        def queue_av_tiles(ms):
            # enqueue col-packed AV matmuls for k-tiles ms; drained a few at a
            # time between QK chunks so PE tracks the ACT/DVE pacemakers
            for m in ms:
                for ch in range(4):
                    pending.append((m, ch))

        out_sb = osb_pool.tile([128, S], F32)

        evac_q = deque()

        def emit_evac():
            while evac_q:
                ch = evac_q.popleft()
                # evac on ACT: DVE carries the staging copies + diag adds
                # (heavier engine); ScalarE reads PSUM slightly faster
                nc.scalar.copy(out_sb[:, ch], o_ps[:, ch])
                # split the two output DMAs across queues: 8 back-to-back
                # ot DMAs on one queue serialize ~6us into the tail
                nc.gpsimd.dma_start(ot[bh[0]][:, ch],
                                    out_sb[0:64, ch])
                nc.sync.dma_start(ot[bh[1]][:, ch],
                                  out_sb[64:128, ch])

        def drain_pending(k=2):
            for _ in range(k):
                if not pending:
                    emit_evac()
                    return
                # don't pop an AV chunk whose vsc isn't emitted yet
                if any(vsc_tiles[s][pending[0][0]] is None for s in AB):
                    return
                m, h = pending.popleft()
                ch = slice(h * 512, (h + 1) * 512)
                for s in AB:
                    # bh A -> out partitions 0-63, bh B -> 64-127
                    nc.tensor.matmul(
                        o_ps[64 * s:64 * s + 64, ch],
                        lhsT=vsc_tiles[s][m][:],
                        rhs=e_tiles[s][m][:, ch],
                        start=(m == 0),
                        stop=(m == NT - 1),
                        skip_group_check=True,
                    )
                if m == NT - 1:
                    # last accumulation for this q-chunk: queue the
                    # evacuation copy + DMA, emitted AFTER the next drain
                    # batch's matmuls so the tail doesn't alternate
                    # PE-MM / DVE-copy serially.
                    evac_q.append(ch)
            emit_evac()

        def emit_vsc(ms):
            # r_inv on DVE (cheap [128,few] op); vsc[m] = val * r_inv[m] on
            # ACT as a Copy with a per-partition scale (same act table as
            # Exp, no table switch). Pool/gpsimd compute ops are avoided
            # entirely: their real Q7 launch overhead is micro-seconds, not
            # the ~100ns the cost model charges.
            if not ms:
                return
            for s in AB:
                nc.vector.reciprocal_approx_fast(
                    r_inv[s][:, ms[0]:ms[-1] + 1],
                    r_sum[s][:, ms[0]:ms[-1] + 1])
            for m in ms:
                for s in AB:
                    vsc = vsc_pool.tile([128, 64], BF16, tag=f"vsc{s}",
                                        name=f"vsc{s}")
                    vsc_tiles[s][m] = vsc
                    nc.vector.tensor_scalar_mul(
                        vsc[:], v_sb[s][:, m * 64:(m + 1) * 64],
                        r_inv[s][:, m:m + 1],
                    )

        # vsc for tile m is emitted inside tile VSC_AT[m] (after that tile's
        # diag units, so Pool never gates the diag->exp chain); its AV
        # matmuls are queued one tile later so PE never waits on fresh vsc.
        # m=14 can be vsc'd/drained during n=15 only when n=14's exps happen
        # in-loop (fully direct); a staged n=14 wide is flushed after n=15
        # and would stall PE's AV(14) matmuls mid-loop.
        tail14 = all(_is_direct(p, 14, s) for s in AB)
        VSC_AT = {4: [0, 1, 2, 3], 8: [4, 5, 6, 7], 12: [8, 9, 10, 11],
                  14: [12], 15: [13, 14] if tail14 else [13]}
        QUEUE_AT = {5: [0, 1, 2, 3], 9: [4, 5, 6, 7], 13: [8, 9, 10, 11],
                    14: [12], 15: [13, 14] if tail14 else [13]}
        TAIL_MS = [15] if tail14 else [14, 15]

        # Wide (staged) exps are deferred by one full n-iteration: ACT's
        # queue is strict FIFO, so a wide exp whose stage input isn't copied
        # yet blocks later PSUM-freeing direct exps and stalls PE+DVE. With
        # a one-iteration delay the staging copies are certainly done and
        # ACT never idles mid-queue.
        deferred_wides = []
        deferred_prev = []

        def flush_prev_wides():
            nonlocal deferred_prev
            for fn in deferred_prev:
                fn()
            deferred_prev = []

        def flush_wides(all_pending=False):
            nonlocal deferred_prev
            flush_prev_wides()
            deferred_prev = deferred_wides[:]
            deferred_wides.clear()
            if all_pending:
                flush_prev_wides()

        def emit_r_merge_and_vsc(ms):
            # issue-order correctness: the reciprocal below READS r_sum[m];
            # a deferred wide exp for m issued later would leave the read
            # uninitialized (Tile deps follow issue order). Flush the
            # previous iteration's wides first — they cover every staged
            # m <= n-1 consumed here.
            if ms:
                flush_prev_wides()
            for m in ms:
                for s in AB:
                    if _is_direct(p, m, s) or _is_x(p, m, s):
                        nc.vector.tensor_add(
                            r_sum[s][:, m:m + 1], r_all[s][:, 0, m:m + 1],
                            r_all[s][:, 1, m:m + 1]
                        )
            emit_vsc(ms)

        for n in range(NT):
            hd = n // 8               # q-half containing this tile's diagonal
            cd = (n % 8) * 128        # diag column offset within that half
            queue_av_tiles(QUEUE_AT.get(n, []))
            for s in AB:
                e_tiles[s][n] = e_pool.tile([128, S], BF16, tag=f"e{s}",
                                            name=f"e{s}_{n}")
            stage = [None if _is_direct(p, n, s) else
                     stg_pool.tile(
                         [128, 1024 if _is_x(p, n, s) else S], F32,
                         tag=f"stg{s}" + ("x" if _is_x(p, n, s) else ""),
                         name=f"stage{s}")
                     for s in AB]
            for h in range(2):
                # QK matmuls interleaved across s (row groups 0-63 / 64-127):
                # consecutive same-weight MMs serialize (the repeated
                # LDWEIGHTS must wait for the in-flight MM on the same cells
                # to drain), but alternating row groups lets each LDWEIGHTS
                # pull ahead during the other head's MM — near 2x PE QK rate.
                sts = {}
                for s in AB:
                    sts[s] = st_pool.tile([128, 1024], F32, tag="st",
                                          name=f"st{s}")
                for j in range(2):
                    for s in AB:
                        q0 = h * 1024 + j * 512
                        nc.tensor.matmul(
                            sts[s][:, j * 512:(j + 1) * 512],
                            lhsT=k2[64 * s:64 * s + 64, n * 128:(n + 1) * 128],
                            rhs=q2[64 * s:64 * s + 64, q0:q0 + 512],
                            start=True,
                            stop=True,
                        )
                for s in AB:
                    direct = _is_direct(p, n, s)
                    st = sts[s]
                    if direct:
                        if h == hd:
                            # diag add in PSUM on PE (same trick as staged)
                            nc.tensor.matmul(
                                st[:, cd:cd + 128],
                                lhsT=eyb[:, 0:128],
                                rhs=eyb[:, 128:256],
                                start=False,
                                stop=True,
                                skip_group_check=True,
                            )
                        nc.scalar.activation(
                            e_tiles[s][n][:, h * 1024:(h + 1) * 1024],
                            st[:],
                            mybir.ActivationFunctionType.Exp,
                            accum_out=r_all[s][:, h:h + 1, n:n + 1],
                        )
                    elif _is_x(p, n, s):
                        if h == 0:
                            nc.vector.tensor_copy(
                                stage[s][:, 0:1024], st[:])
                            if hd == 0:
                                nc.vector.tensor_add(
                                    stage[s][:, cd:cd + 128],
                                    stage[s][:, cd:cd + 128],
                                    rg_eye[:])
                        else:
                            if hd == 1:
                                nc.tensor.matmul(
                                    st[:, cd:cd + 128],
                                    lhsT=eyb[:, 0:128],
                                    rhs=eyb[:, 128:256],
                                    start=False,
                                    stop=True,
                                    skip_group_check=True,
                                )
                            # PSUM-freeing exp first; the SBUF-side exp is
                            # deferred with the wides
                            nc.scalar.activation(
                                e_tiles[s][n][:, 1024:2048],
                                st[:],
                                mybir.ActivationFunctionType.Exp,
                                accum_out=r_all[s][:, 1:2, n:n + 1],
                            )

                            def _x_exp(s=s, n=n, stg=stage[s]):
                                nc.scalar.activation(
                                    e_tiles[s][n][:, 0:1024],
                                    stg[:, 0:1024],
                                    mybir.ActivationFunctionType.Exp,
                                    accum_out=r_all[s][:, 0:1, n:n + 1],
                                )
                            deferred_wides.append(_x_exp)
                    else:
                        # staged: diag added on the SBUF stage by DVE after
                        # the copy — keeps PE's fp16 QK weight stream free of
                        # eyb (bf16) LDWEIGHTS churn
                        nc.vector.tensor_copy(
                            stage[s][:, h * 1024:(h + 1) * 1024], st[:])
                        if h == hd:
                            gc = hd * 1024 + cd
                            nc.vector.tensor_add(
                                stage[s][:, gc:gc + 128],
                                stage[s][:, gc:gc + 128],
                                rg_eye[:])
                        if h == 1:
                            def _wide_exp(s=s, n=n, stg=stage[s]):
                                nc.scalar.activation(
                                    e_tiles[s][n][:],
                                    stg[:],
                                    mybir.ActivationFunctionType.Exp,
                                    accum_out=r_sum[s][:, n:n + 1],
                                )
                            deferred_wides.append(_wide_exp)
                    if s == 1 and h == hd:
                        emit_r_merge_and_vsc(VSC_AT.get(n, []))
                    # drain AV in moderate batches (twice per n): big
                    # once-per-n bursts starve the QK/consumer pipeline
                    # (+44us on HW); 1-2 MM dribbles keep HAM cold
                    if s == 1:
                        drain_pending(2 if n < 6 else (4 if n < 15 else 6))
            flush_wides(all_pending=(n == NT - 1))
        emit_r_merge_and_vsc(TAIL_MS)
        queue_av_tiles(TAIL_MS)
        while pending:
            drain_pending(4)


_NC_CACHE = {}


def build_nc(repeats=1):
    key = repeats
    if key in _NC_CACHE:
        return _NC_CACHE[key]
    nc = bacc.Bacc("TRN2", target_bir_lowering=False, debug=False)
    qt = nc.dram_tensor("qt", [BH_PER_CORE // 2, 2 * D, S], F16, kind="ExternalInput").ap()
    kt = nc.dram_tensor("kt", [BH_PER_CORE // 2, 2 * D, S], F16, kind="ExternalInput").ap()
    vs = nc.dram_tensor("vs", [BH_PER_CORE, 128, NT * 64], F32, kind="ExternalInput").ap()
    rg = nc.dram_tensor("rg", [128, 128], F32, kind="ExternalInput").ap()
    rgb = nc.dram_tensor("rgb", [128, 256], BF16, kind="ExternalInput").ap()
    ot = nc.dram_tensor("ot", [BH_PER_CORE, D, S], F32, kind="ExternalOutput").ap()
    with tile.TileContext(nc) as tc, ExitStack() as ctx:
        if repeats == 1:
            _build_kernel(nc, tc, ctx, qt, kt, vs, rg, rgb, ot)
        else:
            # benchmarking mode: repeat the whole kernel body in an on-device
            # loop so per-iteration time can be extracted from wall clock.
            # A pre-loop exp forces the ACT Exp table load OUTSIDE the loop
            # (otherwise walrus re-inserts the ~2.7us PSEUDO_LOAD per
            # iteration).
            pre_pool = ctx.enter_context(tc.tile_pool(name="pre", bufs=1))
            pre = pre_pool.tile([128, 2], F32)
            nc.sync.dma_start(pre[:, 0:1], rg[:, 0:1])
            nc.scalar.activation(pre[:, 1:2], pre[:, 0:1],
                                 mybir.ActivationFunctionType.Exp)
            with tc.For_i(0, repeats, 1,
                          hint_engines=(mybir.EngineType.PE,
                                        mybir.EngineType.Activation,
                                        mybir.EngineType.DVE)):
                _build_kernel(nc, tc, ctx, qt, kt, vs, rg, rgb, ot)
    nc.compile()
    _NC_CACHE[key] = nc
    return nc


def _prep_inputs(qry, key, val, reg):
    """Host-side shard + layout prep. Returns per-core input maps."""
    q = np.ascontiguousarray(np.asarray(qry, dtype=np.float32)).reshape(BH, S, D)
    k = np.ascontiguousarray(np.asarray(key, dtype=np.float32)).reshape(BH, S, D)
    v = np.ascontiguousarray(np.asarray(val, dtype=np.float32)).reshape(BH, S, D)
    rg = (np.eye(128, dtype=np.float32) * np.float32(np.asarray(reg)))
    import ml_dtypes
    rgb = np.concatenate([np.eye(128, dtype=np.float32),
                          np.eye(128, dtype=np.float32)
                          * np.float32(np.asarray(reg))],
                         axis=1).astype(ml_dtypes.bfloat16)

    in_maps = []
    for c in range(N_CORES):
        sl = slice(c * BH_PER_CORE, (c + 1) * BH_PER_CORE)
        qt = np.ascontiguousarray(
            q[sl].transpose(0, 2, 1).reshape(BH_PER_CORE // 2, 2 * D, S)
        ).astype(np.float16)                                          # [2, 128, S]
        kt = np.ascontiguousarray(
            k[sl].transpose(0, 2, 1).reshape(BH_PER_CORE // 2, 2 * D, S)
        ).astype(np.float16)                                          # [2, 128, S]
        vv = v[sl].reshape(BH_PER_CORE, NT, 128, D)
        vs = np.ascontiguousarray(vv.transpose(0, 2, 1, 3)).reshape(
            BH_PER_CORE, 128, NT * D)                                 # [4, 128, 1024]
        in_maps.append({"qt": qt, "kt": kt, "vs": vs, "rg": rg, "rgb": rgb})
    return in_maps


def kernel(qry, key, val, reg):
    nc = build_nc()
    in_maps = _prep_inputs(qry, key, val, reg)
    res = run_bass_kernel_spmd(nc, in_maps, list(range(N_CORES)))
    out = np.empty((BH, S, D), dtype=np.float32)
    for c in range(N_CORES):
        ot = res.results[c]["ot"]                                    # [4, 64, S]
        for i in range(BH_PER_CORE):
            out[c * BH_PER_CORE + i] = ot[i].T
    return out.reshape(B, H, S, D)

